# revision 1
# baseline (speedup 1.0000x reference)
"""Causal self-attention (B=2, T=2048, C=1024, H=16) on 8 trn2 NeuronCores.

Sharding: core c = (batch b = c // 4, head-group g = c % 4). Each core
computes, for its batch, QKV for heads [4g, 4g+4), causal attention, and a
partial output projection through rows [256g, 256g+256) of W_proj. The host
sums the 4 partial projections per batch (tensor-parallel unshard) and adds
b_proj.

Per-core kernel structure (all matmul inputs bf16, fp32 PSUM):
  - qk^T is produced transposed ([channel, t]) so attention scores need no
    input transposes (contraction over d=64 sits on the partition axis).
  - Scores are computed TRANSPOSED (S^T[k, q] tiles): exp(S^T) is directly
    the P^T operand the PV matmul needs — no PE transposes (which would not
    count as HAM activity and would keep the array clock-throttled).
  - The two heads of a pair are row-packed per S^T matmul (K=64 at array
    rows 0/64) into one 2-bank PSUM tile, so one ACT op handles both exps.
  - ACT (exp) is the bottleneck engine, so everything else is arranged
    around keeping it busy: score matmuls run two k-blocks ahead of the PV
    matmuls (software pipeline), and QKV/V/projection matmul groups are
    emitted as fillers BETWEEN attention steps (the causal structure means
    attention chunk qc only needs QKV chunks <= qc, so chunk qc+1 computes
    while chunk qc's attention runs).
  - Causal masking is a 0/1 multiply on P^T on the otherwise-idle GpSimd.
  - V carries an appended ones column (lhsT [128, 65]) so the PV matmul
    accumulates the softmax denominator as row 64 of y^T_aug for free.
  - Normalization: reciprocal of the denominator row, DMA-broadcast of it
    (stride-0 free dim), multiply fused into the PSUM->SBUF copy.
  - y lands transposed ([d, q]), exactly the lhsT the projection needs.
"""

import sys
from collections import deque

for _p in ("/opt/trn_rl_repo",):
    if _p not in sys.path:
        sys.path.insert(0, _p)

import numpy as np
import ml_dtypes

import concourse.bass as bass
import concourse.tile as tile
from concourse import bacc, mybir
from concourse.bass_utils import run_bass_kernel_spmd

BF16 = mybir.dt.bfloat16
F32 = mybir.dt.float32
NP_BF16 = ml_dtypes.bfloat16

B, T, C = 2, 2048, 1024
H, D = 16, 64
N_CORES = 8
CT = C // 128   # 8 contraction tiles
TQ = T // 128   # 16 key blocks
QC = T // 512   # 4 query chunks
SCALE = 1.0 / np.sqrt(D)

_compiled = None


def _build_nc(dbg=False):
    nc = bacc.Bacc("TRN2", target_bir_lowering=False, debug=False,
                   enable_asserts=False)
    if dbg:
        dbg_yT = nc.dram_tensor("dbg_yT", [128, 2, T], BF16, kind="ExternalOutput")

    xT_d = nc.dram_tensor("xT", [QC, C, 512], BF16, kind="ExternalInput")
    wqk_d = nc.dram_tensor("wqk", [C, 512], BF16, kind="ExternalInput")
    wv_d = nc.dram_tensor("wv", [C, 256], BF16, kind="ExternalInput")
    wp_d = nc.dram_tensor("wp", [256, C], BF16, kind="ExternalInput")
    bqk_d = nc.dram_tensor("bqk", [128, 4], F32, kind="ExternalInput")
    bv_d = nc.dram_tensor("bv", [128, 256], BF16, kind="ExternalInput")
    mask_d = nc.dram_tensor("maskb", [128, 4, 512], BF16, kind="ExternalInput")
    out_d = nc.dram_tensor("out", [T, C], F32, kind="ExternalOutput")

    Exp = mybir.ActivationFunctionType.Exp

    with tile.TileContext(nc) as tc:
        with (
            tc.tile_pool(name="const", bufs=1) as cpool,
            tc.tile_pool(name="qkT", bufs=1) as qkpool,
            tc.tile_pool(name="vbuf", bufs=1) as vpool,
            tc.tile_pool(name="ybuf", bufs=1) as ypool,
            tc.tile_pool(name="pt", bufs=5) as ptpool,
            tc.tile_pool(name="norm", bufs=8) as npool,
            tc.tile_pool(name="ostage", bufs=3) as opool,
            tc.tile_pool(name="mmps", bufs=4, space="PSUM") as mmps,
            tc.tile_pool(name="sps", bufs=2, space="PSUM") as sps,
        ):
            # ---- constants / weights ----
            xT_s = cpool.tile([128, CT, T], BF16)
            wqk_s = cpool.tile([128, CT, 512], BF16)
            wv_s = cpool.tile([128, CT, 256], BF16)
            wp_s = cpool.tile([128, 2, C], BF16)
            bqk_s = cpool.tile([128, 4], F32)
            bv_s = cpool.tile([128, 256], BF16)
            mask_s = cpool.tile([128, 4, 512], BF16)

            warm = cpool.tile([128, 1], F32)
            nc.vector.memset(warm[:], 0.0)
            nc.scalar.activation(warm[:], warm[:], Exp)

            for i in range(CT):
                nc.sync.dma_start(out=wqk_s[:, i, :], in_=wqk_d.ap()[128 * i:128 * (i + 1), :])
            nc.sync.dma_start(out=bqk_s[:], in_=bqk_d.ap()[:])
            nc.sync.dma_start(out=mask_s[:], in_=mask_d.ap()[:])
            for i in range(CT):
                nc.sync.dma_start(out=wv_s[:, i, :], in_=wv_d.ap()[128 * i:128 * (i + 1), :])
            nc.sync.dma_start(out=bv_s[:], in_=bv_d.ap()[:])
            for i in range(2):
                nc.sync.dma_start(out=wp_s[:, i, :], in_=wp_d.ap()[128 * i:128 * (i + 1), :])

            qkT_s = qkpool.tile([128, 4, T], BF16)
            v_s = vpool.tile([128, TQ, 4, 65], BF16)
            nc.vector.memset(v_s[:, :, :, 64:65], 1.0)
            yT_s = ypool.tile([128, 2, T], BF16)

            # ---- emission helpers (work groups used directly or as fillers) ----
            def dma_chunk(t4):
                for i in range(CT):
                    nc.sync.dma_start(out=xT_s[:, i, 512 * t4:512 * (t4 + 1)],
                                      in_=xT_d.ap()[t4, 128 * i:128 * (i + 1), :])

            def qkv_group(j, t4):
                # jtile 0: Q heads {0,1}; 1: Q {2,3}; 2: K {0,1}; 3: K {2,3}
                ps = mmps.tile([128, 512], F32, tag="mm")
                for i in range(CT):
                    nc.tensor.matmul(
                        ps[:],
                        wqk_s[:, i, 128 * j:128 * (j + 1)],
                        xT_s[:, i, 512 * t4:512 * (t4 + 1)],
                        start=(i == 0), stop=(i == CT - 1),
                    )
                nc.vector.tensor_scalar_add(
                    qkT_s[:, j, 512 * t4:512 * (t4 + 1)], ps[:], bqk_s[:, j:j + 1])

            def v_group(t):
                ps = mmps.tile([128, 256], F32, tag="mm")
                for i in range(CT):
                    nc.tensor.matmul(
                        ps[:],
                        xT_s[:, i, 128 * t:128 * (t + 1)],
                        wv_s[:, i, :],
                        start=(i == 0), stop=(i == CT - 1),
                    )
                nc.vector.tensor_add(
                    v_s[:, t, :, 0:64],
                    ps[:].rearrange("p (h d) -> p h d", h=4),
                    bv_s[:].rearrange("p (h d) -> p h d", h=4))

            def proj_group(t):
                o_t = opool.tile([128, C], F32, tag="o")
                for n in range(2):
                    ps = mmps.tile([128, 512], F32, tag="mm")
                    for p2 in range(2):
                        nc.tensor.matmul(
                            ps[:],
                            yT_s[:, p2, 128 * t:128 * (t + 1)],
                            wp_s[:, p2, 512 * n:512 * (n + 1)],
                            start=(p2 == 0), stop=(p2 == 1),
                        )
                    nc.vector.tensor_copy(o_t[:, 512 * n:512 * (n + 1)], ps[:])
                nc.sync.dma_start(out=out_d.ap()[128 * t:128 * (t + 1), :], in_=o_t[:])

            fillers = deque()

            def emit_filler(n=1):
                for _ in range(n):
                    if fillers:
                        fillers.popleft()()

            # ---- prologue: first input chunk + pair-0's QKV j-tiles + V ----
            dma_chunk(0)
            for j in (0, 2):
                qkv_group(j, 0)
            for t in range(4):
                v_group(t)

            # ---- attention: S^T tiles [k-block, q-chunk], flash over k ----
            for qc in range(QC):
                # stage work for the NEXT chunk + projection of the PREVIOUS
                if qc == 0:
                    for j in (1, 3):
                        fillers.append(lambda j=j: qkv_group(j, 0))
                if qc + 1 < QC:
                    fillers.append(lambda t4=qc + 1: dma_chunk(t4))
                    for j in range(4):
                        fillers.append(lambda j=j, t4=qc + 1: qkv_group(j, t4))
                    for t in range(4 * (qc + 1), 4 * (qc + 2)):
                        fillers.append(lambda t=t: v_group(t))
                if qc >= 1:
                    for t in range(4 * (qc - 1), 4 * qc):
                        fillers.append(lambda t=t: proj_group(t))

                for p in range(2):
                    jq, jk = p, 2 + p
                    nkb = 4 * qc + 4
                    ya = mmps.tile([65, 512], F32, tag="mm")
                    yb = mmps.tile([65, 512], F32, tag="mm")
                    pts = {}

                    def stage(kb, qc=qc, jq=jq, jk=jk, pts=None):
                        """score matmuls + exp (+ causal 0/1 mask on gpsimd)"""
                        m = kb - 4 * qc  # >= 0 on the diagonal chunk
                        s_ps = sps.tile([128, 1024], F32, tag="spair")
                        for hi, part in ((0, slice(0, 64)), (1, slice(64, 128))):
                            nc.tensor.matmul(
                                s_ps[:, 512 * hi:512 * (hi + 1)],
                                qkT_s[part, jk, 128 * kb:128 * (kb + 1)],
                                qkT_s[part, jq, 512 * qc:512 * (qc + 1)],
                                start=True, stop=True,
                                tile_position=(64 * hi, 0), skip_group_check=True)
                        pt = ptpool.tile([128, 1024], BF16, tag="pt")
                        nc.scalar.activation(pt[:], s_ps[:], Exp, scale=SCALE)
                        if m >= 0:
                            wm = 128 * (m + 1)
                            for hi in range(2):
                                nc.gpsimd.tensor_mul(
                                    pt[:, 512 * hi:512 * hi + wm],
                                    pt[:, 512 * hi:512 * hi + wm],
                                    mask_s[:, m, 0:wm])
                        pts[kb] = pt

                    def consume(kb, p=p, ya=ya, yb=yb, nkb=nkb, pts=None):
                        pt = pts.pop(kb)
                        for hi, y_ps in ((0, ya), (1, yb)):
                            nc.tensor.matmul(
                                y_ps[:],
                                v_s[:, kb, 2 * p + hi, :],
                                pt[:, 512 * hi:512 * (hi + 1)],
                                start=(kb == 0), stop=(kb == nkb - 1))

                    DEPTH = 2
                    for kb in range(min(DEPTH, nkb)):
                        stage(kb, pts=pts)
                    for kb in range(nkb):
                        if kb + DEPTH < nkb:
                            stage(kb + DEPTH, pts=pts)
                        consume(kb, pts=pts)
                        emit_filler(1)

                    # normalize + write y^T (head A -> partitions 0:64, B -> 64:128).
                    # Copies run inline (they free the accumulator PSUM slots);
                    # the broadcast DMAs + reciprocal + multiply are deferred
                    # into the next attention steps so the DMA latency never
                    # head-blocks the in-order DVE queue.
                    ycs, lbs = [], []
                    for y_ps in (ya, yb):
                        yc = npool.tile([65, 512], F32, tag="yc")
                        nc.vector.tensor_copy(yc[:], y_ps[:])
                        ycs.append(yc)
                        lb = npool.tile([64, 512], F32, tag="lb")
                        lbs.append(lb)

                    def norm_dma(ycs=ycs, lbs=lbs):
                        for yc, lb in zip(ycs, lbs):
                            nc.sync.dma_start(
                                out=lb[:],
                                in_=yc[64:65, :].unsqueeze(1).broadcast_to([1, 64, 512]))

                    def norm_mul(hi, p=p, qc=qc, ycs=ycs, lbs=lbs):
                        rb = npool.tile([64, 512], F32, tag="rb")
                        nc.vector.reciprocal_approx_fast(rb[:], lbs[hi][:])
                        nc.vector.tensor_mul(
                            yT_s[64 * hi:64 * (hi + 1), p, 512 * qc:512 * (qc + 1)],
                            ycs[hi][0:64, :], rb[:])

                    fillers.appendleft(lambda f0=norm_mul: (f0(0), f0(1)))
                    fillers.appendleft(norm_dma)

            if dbg:
                nc.sync.dma_start(out=dbg_yT.ap()[:], in_=yT_s[:])

            # ---- epilogue: leftover fillers + final projection chunk ----
            emit_filler(len(fillers))
            for t in range(4 * (QC - 1), TQ):
                proj_group(t)

    nc.compile()
    return nc


def _shard_inputs(x, W_attn, b_attn, W_proj, b_proj):
    """Build the 8 per-core input maps (numpy, bf16 where applicable)."""
    # maskb[:, m, :]: 0/1 keep-mask for the diagonal k-block at offset 128m
    # within the q-chunk: S^T entry (p, j) masked (0) where 128m + p > j.
    pp = np.arange(128)[:, None]
    jj = np.arange(512)[None, :]
    maskb = np.stack(
        [np.where(128 * m + pp > jj, 0.0, 1.0) for m in range(4)],
        axis=1).astype(NP_BF16)  # [128, 4, 512]
    in_maps = []
    for c in range(N_CORES):
        b, g = c // 4, c % 4
        ch = slice(256 * g, 256 * (g + 1))
        wq = W_attn[:, ch]
        wk = W_attn[:, C:][:, ch]
        wv = W_attn[:, 2 * C:][:, ch]
        wqk = np.concatenate([wq, wk], axis=1).astype(NP_BF16)
        bq = b_attn[ch]
        bk = b_attn[C:][ch]
        bv = b_attn[2 * C:][ch]
        bqk = np.concatenate([bq, bk]).reshape(4, 128).T.astype(np.float32)  # [128, 4]
        xTc = np.ascontiguousarray(
            x[b].T.reshape(C, QC, 512).transpose(1, 0, 2)).astype(NP_BF16)
        in_maps.append({
            "xT": xTc,
            "wqk": wqk,
            "wv": wv.astype(NP_BF16),
            "wp": W_proj[ch, :].astype(NP_BF16),
            "bqk": np.ascontiguousarray(bqk),
            "bv": np.broadcast_to(bv.astype(NP_BF16), (128, 256)).copy(),
            "maskb": maskb,
        })
    return in_maps


def _run(in_maps, trace=False, **kw):
    global _compiled
    if _compiled is None:
        _compiled = _build_nc()
    return run_bass_kernel_spmd(_compiled, in_maps, list(range(N_CORES)),
                                trace=trace, **kw)


def kernel(x, W_attn, b_attn, W_proj, b_proj):
    x = np.asarray(x, dtype=np.float32)
    W_attn = np.asarray(W_attn, dtype=np.float32)
    b_attn = np.asarray(b_attn, dtype=np.float32)
    W_proj = np.asarray(W_proj, dtype=np.float32)
    b_proj = np.asarray(b_proj, dtype=np.float32)

    in_maps = _shard_inputs(x, W_attn, b_attn, W_proj, b_proj)
    res = _run(in_maps)
    out = np.zeros((B, T, C), dtype=np.float32)
    for c in range(N_CORES):
        out[c // 4] += res.results[c]["out"]
    out += b_proj
    return out



# revision 11
# speedup vs baseline: 1.2250x; 1.2250x over previous
"""Causal self-attention (B=2, T=2048, C=1024, H=16) on 8 trn2 NeuronCores.

Sharding: core c = (batch b = c // 4, head-group g = c % 4). Each core
computes, for its batch, QKV for heads [4g, 4g+4), causal attention, and a
partial output projection through rows [256g, 256g+256) of W_proj. The host
sums the 4 partial projections per batch (tensor-parallel unshard) and adds
b_proj.

Per-core kernel structure (all matmul inputs bf16, fp32 PSUM):
  - qk^T is produced transposed ([channel, t]) so attention scores need no
    input transposes (contraction over d=64 sits on the partition axis).
  - Scores are computed TRANSPOSED (S^T[k, q] tiles): exp(S^T) is directly
    the P^T operand the PV matmul needs. The two heads of a pair go to the
    two halves of a [128, 2, 512] PSUM tile (one bank per head) via
    row-packed K=64 matmuls at tile_position rows 0/64.
  - Causal masking costs NO cross-engine sync: for diagonal k-blocks a
    third matmul accumulates identity.T @ (-30000 * tril_mask) into the
    live 128-column window of the score PSUM (start=False), so exp()
    produces exact zeros there. GpSimd is completely idle.
  - Diagonal k-block tiles are column-restricted to the causally-live
    query range (scores, exp and PV all skip the dead columns).
  - One ACT exp per (pair, k-block) covers both heads ([128, 2, live]).
  - V carries an appended ones column (lhsT [128, 65]) so the PV matmul
    accumulates the softmax denominator as row 64 of y^T_aug for free.
  - Normalization: PSUM->SBUF copy of y_aug (bf16), reciprocal of the
    [1, 1024] denominator row FIRST, then a stride-0 broadcast DMA of the
    reciprocal, then bf16 multiplies (DVE 2x) fused into the yT write.
    recip/broadcast/mul are deferred into later attention steps so DMA
    latency never head-blocks the DVE queue.
  - y lands transposed ([d, q]), exactly the lhsT the projection needs.
    Projection partials are written to DRAM in bf16 (host sums in fp32).
  - PE is the bottleneck engine (~278K matmul columns); QKV/V/projection
    matmul groups are emitted as fillers BETWEEN attention steps, weighted
    toward the early (PE-starved ACT) chunks for QKV/V and the late
    (ACT-bound) chunks for projection, so the PE instruction queue never
    drains (keeps the PE clock in its fast DVFS state).
"""

import sys
from collections import deque

for _p in ("/opt/trn_rl_repo",):
    if _p not in sys.path:
        sys.path.insert(0, _p)

import numpy as np
import ml_dtypes

import concourse.bass as bass
import concourse.tile as tile
from concourse import bacc, mybir
from concourse.bass_utils import run_bass_kernel_spmd

BF16 = mybir.dt.bfloat16
F32 = mybir.dt.float32
NP_BF16 = ml_dtypes.bfloat16

B, T, C = 2, 2048, 1024
H, D = 16, 64
N_CORES = 8
CT = C // 128   # 8 contraction tiles
TQ = T // 128   # 16 key blocks
QC = T // 512   # 4 query chunks
SCALE = 1.0 / np.sqrt(D)
NEG = -240.0  # masked-score bias; exp(SCALE*(S+NEG)) <= e^-24 ~ 4e-11
              # (kept small so the HW exp table input stays in-domain)

_compiled = None


def _build_nc(dbg=False):
    nc = bacc.Bacc("TRN2", target_bir_lowering=False, debug=False,
                   enable_asserts=False)
    if dbg:
        dbg_qkT = nc.dram_tensor("dbg_qkT", [128, 4, T], BF16, kind="ExternalOutput")
        dbg_v = nc.dram_tensor("dbg_v", [128, TQ, 4, 65], BF16, kind="ExternalOutput")
        dbg_yT = nc.dram_tensor("dbg_yT", [128, 2, T], BF16, kind="ExternalOutput")

    xT_d = nc.dram_tensor("xT", [QC, C, 512], BF16, kind="ExternalInput")
    wqk_d = nc.dram_tensor("wqk", [C, 512], BF16, kind="ExternalInput")
    wv_d = nc.dram_tensor("wv", [C, 256], BF16, kind="ExternalInput")
    wp_d = nc.dram_tensor("wp", [256, C], BF16, kind="ExternalInput")
    bqk_d = nc.dram_tensor("bqk", [128, 4], F32, kind="ExternalInput")
    bv_d = nc.dram_tensor("bv", [128, 256], BF16, kind="ExternalInput")
    idn_d = nc.dram_tensor("idn", [128, 128], BF16, kind="ExternalInput")
    mskb_d = nc.dram_tensor("mskb", [128, 128], BF16, kind="ExternalInput")
    out_d = nc.dram_tensor("out", [T, C], BF16, kind="ExternalOutput")

    Exp = mybir.ActivationFunctionType.Exp

    with tile.TileContext(nc) as tc:
        with (
            tc.tile_pool(name="const", bufs=1) as cpool,
            tc.tile_pool(name="qkT", bufs=1) as qkpool,
            tc.tile_pool(name="vbuf", bufs=1) as vpool,
            tc.tile_pool(name="ybuf", bufs=1) as ypool,
            tc.tile_pool(name="pt", bufs=4) as ptpool,
            tc.tile_pool(name="norm", bufs=2) as npool,
            tc.tile_pool(name="ostage", bufs=2) as opool,
            tc.tile_pool(name="mmps", bufs=2, space="PSUM") as mmps,
            tc.tile_pool(name="sps", bufs=2, space="PSUM") as sps,
            tc.tile_pool(name="accps", bufs=2, space="PSUM") as accps,
        ):
            # ---- constants / weights ----
            xT_s = cpool.tile([128, CT, T], BF16)
            wqk_s = cpool.tile([128, CT, 512], BF16)
            wv_s = cpool.tile([128, CT, 256], BF16)
            wp_s = cpool.tile([128, 2, C], BF16)
            bqk_s = cpool.tile([128, 4], F32)
            bv_s = cpool.tile([128, 256], BF16)
            idn_s = cpool.tile([128, 128], BF16)
            mskb_s = cpool.tile([128, 128], BF16)

            warm = cpool.tile([128, 1], F32)
            nc.vector.memset(warm[:], 0.0)
            nc.scalar.activation(warm[:], warm[:], Exp)

            for i in range(CT):
                nc.sync.dma_start(out=xT_s[:, i, 0:512],
                                  in_=xT_d.ap()[0, 128 * i:128 * (i + 1), :])
            for i in range(CT):
                nc.sync.dma_start(out=wqk_s[:, i, :], in_=wqk_d.ap()[128 * i:128 * (i + 1), :])
            nc.sync.dma_start(out=idn_s[:], in_=idn_d.ap()[:])
            nc.sync.dma_start(out=mskb_s[:], in_=mskb_d.ap()[:])
            nc.sync.dma_start(out=bqk_s[:], in_=bqk_d.ap()[:])
            for i in range(CT):
                nc.sync.dma_start(out=wv_s[:, i, :], in_=wv_d.ap()[128 * i:128 * (i + 1), :])
            nc.sync.dma_start(out=bv_s[:], in_=bv_d.ap()[:])
            for i in range(2):
                nc.sync.dma_start(out=wp_s[:, i, :], in_=wp_d.ap()[128 * i:128 * (i + 1), :])

            qkT_s = qkpool.tile([128, 4, T], BF16)
            v_s = vpool.tile([128, TQ, 4, 65], BF16)
            nc.vector.memset(v_s[:, :, :, 64:65], 1.0)
            yT_s = ypool.tile([128, 2, T], BF16)

            # ---- emission helpers (work groups used directly or as fillers) ----
            def dma_chunk(t4):
                for i in range(CT):
                    nc.sync.dma_start(out=xT_s[:, i, 512 * t4:512 * (t4 + 1)],
                                      in_=xT_d.ap()[t4, 128 * i:128 * (i + 1), :])

            def qkv_half(j, t4, h):
                # jtile 0: Q heads {0,1}; 1: Q {2,3}; 2: K {0,1}; 3: K {2,3}
                # emitted as two filler halves (h=0 allocates, h=1 drains)
                ps = qkv_half.ps if h else mmps.tile([128, 512], F32, tag="mm")
                qkv_half.ps = ps
                for i in range(4 * h, 4 * h + 4):
                    nc.tensor.matmul(
                        ps[:],
                        wqk_s[:, i, 128 * j:128 * (j + 1)],
                        xT_s[:, i, 512 * t4:512 * (t4 + 1)],
                        start=(i == 0), stop=(i == CT - 1),
                    )
                if h:
                    nc.vector.tensor_scalar_add(
                        qkT_s[:, j, 512 * t4:512 * (t4 + 1)], ps[:], bqk_s[:, j:j + 1])

            def qkv_group(j, t4):
                qkv_half(j, t4, 0)
                qkv_half(j, t4, 1)

            def v_group(t):
                ps = mmps.tile([128, 256], F32, tag="mm")
                for i in range(CT):
                    nc.tensor.matmul(
                        ps[:],
                        xT_s[:, i, 128 * t:128 * (t + 1)],
                        wv_s[:, i, :],
                        start=(i == 0), stop=(i == CT - 1),
                    )
                nc.vector.tensor_add(
                    v_s[:, t, :, 0:64],
                    ps[:].rearrange("p (h d) -> p h d", h=4),
                    bv_s[:].rearrange("p (h d) -> p h d", h=4))

            def proj_half(t, n, o_t):
                ps = mmps.tile([128, 512], F32, tag="mm")
                for p2 in range(2):
                    nc.tensor.matmul(
                        ps[:],
                        yT_s[:, p2, 128 * t:128 * (t + 1)],
                        wp_s[:, p2, 512 * n:512 * (n + 1)],
                        start=(p2 == 0), stop=(p2 == 1),
                    )
                nc.vector.tensor_copy(o_t[:, 512 * n:512 * (n + 1)], ps[:])
                if n == 1:
                    nc.sync.dma_start(out=out_d.ap()[128 * t:128 * (t + 1), :], in_=o_t[:])

            def proj_group(t):
                o_t = opool.tile([128, C], BF16, tag="o")
                proj_half(t, 0, o_t)
                proj_half(t, 1, o_t)

            NODL = (9, 9)
            fillers = deque()  # (deadline (qc, p), fn)

            def emit_filler(n=1):
                for _ in range(n):
                    if fillers:
                        fillers.popleft()[1]()

            def flush_due(key):
                """Emit every queued filler whose deadline is <= key."""
                keep = deque()
                while fillers:
                    dl, fn = fillers.popleft()
                    if dl <= key:
                        fn()
                    else:
                        keep.append((dl, fn))
                fillers.extend(keep)

            # ---- prologue: pair-0's Q/K j-tiles + V t-block 0..3 ----
            qkv_group(0, 0)
            qkv_group(2, 0)
            for t in range(4):
                v_group(t)

            # ---- attention: S^T tiles [k-block, q-chunk], flash over k ----
            for qc in range(QC):
                # stage work for later chunks (see scheduling notes in header)
                if qc == 0:
                    fillers.append(((0, 1), lambda: qkv_group(1, 0)))
                    fillers.append(((0, 1), lambda: qkv_group(3, 0)))
                if qc + 1 < QC:
                    dl = (qc + 1, 0)
                    fillers.append((dl, lambda t4=qc + 1: dma_chunk(t4)))
                    for j in range(4):
                        fillers.append((dl, lambda j=j, t4=qc + 1: qkv_half(j, t4, 0)))
                        fillers.append((dl, lambda j=j, t4=qc + 1: qkv_half(j, t4, 1)))
                    for t in range(4 * (qc + 1), 4 * (qc + 2)):
                        fillers.append((dl, lambda t=t: v_group(t)))
                # projection: qc0 during qc2; qc1+qc2 during qc3
                for tp in {2: range(0, 4), 3: range(4, 12)}.get(qc, ()):
                    fillers.append((NODL, lambda t=tp: proj_group(t)))

                for p in range(2):
                    flush_due((qc, p))
                    jq, jk = p, 2 + p
                    nkb = 4 * qc + 4
                    ya = accps.tile([65, 512], F32, tag="acc")
                    yb = accps.tile([65, 512], F32, tag="acc")
                    pts = {}

                    def stage(kb, qc=qc, jq=jq, jk=jk, pts=None):
                        """score (+ causal bias) matmuls + exp for both heads"""
                        m = kb - 4 * qc  # >= 0 on the diagonal chunk
                        lv = 128 * max(m, 0)  # first causally-live column
                        s_ps = sps.tile([128, 2, 512], F32, tag="spair")
                        for hi in range(2):
                            nc.tensor.matmul(
                                s_ps[:, hi, lv:512],
                                qkT_s[64 * hi:64 * (hi + 1), jk, 128 * kb:128 * (kb + 1)],
                                qkT_s[64 * hi:64 * (hi + 1), jq, 512 * qc + lv:512 * (qc + 1)],
                                start=True, stop=(m < 0),
                                tile_position=(64 * hi, 0), skip_group_check=True)
                        if m >= 0:
                            for hi in range(2):
                                nc.tensor.matmul(
                                    s_ps[:, hi, lv:lv + 128],
                                    idn_s[:], mskb_s[:],
                                    start=False, stop=True,
                                    tile_position=(0, 0), skip_group_check=True)
                        pt = ptpool.tile([128, 2, 512], BF16, tag="pt")
                        nc.scalar.activation(pt[:, :, lv:512], s_ps[:, :, lv:512],
                                             Exp, scale=SCALE)
                        pts[kb] = (pt, lv)

                    def consume(kb, p=p, ya=ya, yb=yb, nkb=nkb, qc=qc, pts=None):
                        pt, lv = pts.pop(kb)
                        for hi, y_ps in ((0, ya), (1, yb)):
                            nc.tensor.matmul(
                                y_ps[:, lv:512],
                                v_s[:, kb, 2 * p + hi, :],
                                pt[:, hi, lv:512],
                                start=(kb == 0), stop=(kb == nkb - 1),
                                skip_group_check=True)

                    DEPTH = 2
                    for kb in range(min(DEPTH, nkb)):
                        stage(kb, pts=pts)
                    for kb in range(nkb):
                        if kb + DEPTH < nkb:
                            stage(kb + DEPTH, pts=pts)
                        consume(kb, pts=pts)
                        emit_filler(2 if qc == 0 else 1)

                    # normalize + write y^T (head A -> partitions 0:64, B -> 64:128).
                    # Copies run inline (they free the accumulator PSUM banks);
                    # reciprocal -> broadcast DMA -> multiply are deferred into
                    # the next attention steps.
                    yc = npool.tile([65, 2, 512], F32, tag="yc")
                    nc.vector.tensor_copy(yc[:, 0, :], ya[:])
                    nc.vector.tensor_copy(yc[:, 1, :], yb[:])
                    rb = npool.tile([64, 2, 512], F32, tag="rb")
                    rr = npool.tile([64, 2, 512], F32, tag="rr")

                    def norm_bcast(yc=yc, rb=rb):
                        # DMA (not DVE) reads the partition-64 denominator row:
                        # DVE input partition offsets are unreliable on HW.
                        nc.sync.dma_start(
                            out=rb[:],
                            in_=yc[64:65, :, :].unsqueeze(1)
                                .broadcast_to([1, 64, 2, 512]))

                    def norm_recip(rb=rb, rr=rr):
                        nc.vector.reciprocal_approx_fast(rr[:], rb[:])

                    def norm_mul(p=p, qc=qc, yc=yc, rr=rr):
                        for hi in range(2):
                            nc.vector.tensor_mul(
                                yT_s[64 * hi:64 * (hi + 1), p, 512 * qc:512 * (qc + 1)],
                                yc[0:64, hi, :], rr[:, hi, :])

                    fillers.appendleft((NODL, norm_mul))
                    fillers.appendleft((NODL, norm_recip))
                    fillers.appendleft((NODL, norm_bcast))

            # ---- epilogue: leftover fillers + final projection chunk ----
            emit_filler(len(fillers))
            if dbg:
                nc.sync.dma_start(out=dbg_qkT.ap()[:], in_=qkT_s[:])
                nc.sync.dma_start(out=dbg_v.ap()[:], in_=v_s[:])
                nc.sync.dma_start(out=dbg_yT.ap()[:], in_=yT_s[:])
            for t in range(4 * (QC - 1), TQ):
                proj_group(t)

    nc.compile()
    return nc


def _shard_inputs(x, W_attn, b_attn, W_proj, b_proj):
    """Build the 8 per-core input maps (numpy, bf16 where applicable)."""
    pp = np.arange(128)[:, None]
    jj = np.arange(128)[None, :]
    mskb = np.where(pp > jj, NEG, 0.0).astype(NP_BF16)  # [128, 128]
    idn = np.eye(128, dtype=NP_BF16)
    in_maps = []
    for c in range(N_CORES):
        b, g = c // 4, c % 4
        ch = slice(256 * g, 256 * (g + 1))
        wq = W_attn[:, ch]
        wk = W_attn[:, C:][:, ch]
        wv = W_attn[:, 2 * C:][:, ch]
        wqk = np.concatenate([wq, wk], axis=1).astype(NP_BF16)
        bq = b_attn[ch]
        bk = b_attn[C:][ch]
        bv = b_attn[2 * C:][ch]
        bqk = np.concatenate([bq, bk]).reshape(4, 128).T.astype(np.float32)  # [128, 4]
        xTc = np.ascontiguousarray(
            x[b].T.reshape(C, QC, 512).transpose(1, 0, 2)).astype(NP_BF16)
        in_maps.append({
            "xT": xTc,
            "wqk": wqk,
            "wv": wv.astype(NP_BF16),
            "wp": W_proj[ch, :].astype(NP_BF16),
            "bqk": np.ascontiguousarray(bqk),
            "bv": np.broadcast_to(bv.astype(NP_BF16), (128, 256)).copy(),
            "idn": idn,
            "mskb": mskb,
        })
    return in_maps


def _run(in_maps, trace=False, **kw):
    global _compiled
    if _compiled is None:
        _compiled = _build_nc()
    return run_bass_kernel_spmd(_compiled, in_maps, list(range(N_CORES)),
                                trace=trace, **kw)


def kernel(x, W_attn, b_attn, W_proj, b_proj):
    x = np.asarray(x, dtype=np.float32)
    W_attn = np.asarray(W_attn, dtype=np.float32)
    b_attn = np.asarray(b_attn, dtype=np.float32)
    W_proj = np.asarray(W_proj, dtype=np.float32)
    b_proj = np.asarray(b_proj, dtype=np.float32)

    in_maps = _shard_inputs(x, W_attn, b_attn, W_proj, b_proj)
    res = _run(in_maps)
    out = np.zeros((B, T, C), dtype=np.float32)
    for c in range(N_CORES):
        out[c // 4] += np.asarray(res.results[c]["out"], dtype=np.float32)
    out += b_proj
    return out


# revision 15
# speedup vs baseline: 1.3053x; 1.0655x over previous
"""Causal self-attention (B=2, T=2048, C=1024, H=16) on 8 trn2 NeuronCores.

Sharding: core c = (batch b = c // 4, head-group g = c % 4). Each core
computes, for its batch, QKV for heads [4g, 4g+4), causal attention, and a
partial output projection through rows [256g, 256g+256) of W_proj. The host
sums the 4 partial projections per batch (tensor-parallel unshard) and adds
b_proj.

Per-core kernel structure (all matmul inputs bf16, fp32 PSUM):
  - qk^T is produced transposed ([channel, t]) so attention scores need no
    input transposes (contraction over d=64 sits on the partition axis).
  - Scores are computed TRANSPOSED (S^T[k, q] tiles): exp(S^T) is directly
    the P^T operand the PV matmul needs. The two heads of a pair go to the
    two halves of a [128, 2, 512] PSUM tile (one bank per head) via
    row-packed K=64 matmuls at tile_position rows 0/64.
  - Causal masking costs NO cross-engine sync: for diagonal k-blocks a
    third matmul accumulates identity.T @ (-30000 * tril_mask) into the
    live 128-column window of the score PSUM (start=False), so exp()
    produces exact zeros there. GpSimd is completely idle.
  - Diagonal k-block tiles are column-restricted to the causally-live
    query range (scores, exp and PV all skip the dead columns).
  - One ACT exp per (pair, k-block) covers both heads ([128, 2, live]).
  - V carries an appended ones column (lhsT [128, 65]) so the PV matmul
    accumulates the softmax denominator as row 64 of y^T_aug for free.
  - Normalization: PSUM->SBUF copy of y_aug (bf16), reciprocal of the
    [1, 1024] denominator row FIRST, then a stride-0 broadcast DMA of the
    reciprocal, then bf16 multiplies (DVE 2x) fused into the yT write.
    recip/broadcast/mul are deferred into later attention steps so DMA
    latency never head-blocks the DVE queue.
  - y lands transposed ([d, q]), exactly the lhsT the projection needs.
    Projection partials are written to DRAM in bf16 (host sums in fp32).
  - PE is the bottleneck engine (~278K matmul columns); QKV/V/projection
    matmul groups are emitted as fillers BETWEEN attention steps, weighted
    toward the early (PE-starved ACT) chunks for QKV/V and the late
    (ACT-bound) chunks for projection, so the PE instruction queue never
    drains (keeps the PE clock in its fast DVFS state).
"""

import sys
from collections import deque

for _p in ("/opt/trn_rl_repo",):
    if _p not in sys.path:
        sys.path.insert(0, _p)

import numpy as np
import ml_dtypes

import concourse.bass as bass
import concourse.tile as tile
from concourse import bacc, mybir
from concourse.bass_utils import run_bass_kernel_spmd

BF16 = mybir.dt.bfloat16
F32 = mybir.dt.float32
NP_BF16 = ml_dtypes.bfloat16

B, T, C = 2, 2048, 1024
H, D = 16, 64
N_CORES = 8
CT = C // 128   # 8 contraction tiles
TQ = T // 128   # 16 key blocks
QC = T // 512   # 4 query chunks
SCALE = 1.0 / np.sqrt(D)
NEG = -240.0  # masked-score bias; exp(SCALE*(S+NEG)) <= e^-24 ~ 4e-11
              # (kept small so the HW exp table input stays in-domain)

_compiled = None


def _build_nc(dbg=False):
    nc = bacc.Bacc("TRN2", target_bir_lowering=False, debug=False,
                   enable_asserts=False)
    if dbg:
        dbg_qkT = nc.dram_tensor("dbg_qkT", [128, 4, T], BF16, kind="ExternalOutput")
        dbg_v = nc.dram_tensor("dbg_v", [128, TQ, 4, 65], BF16, kind="ExternalOutput")
        dbg_yT = nc.dram_tensor("dbg_yT", [128, 2, T], BF16, kind="ExternalOutput")

    xT_d = nc.dram_tensor("xT", [QC, C, 512], BF16, kind="ExternalInput")
    wqk_d = nc.dram_tensor("wqk", [C, 512], BF16, kind="ExternalInput")
    wv_d = nc.dram_tensor("wv", [C, 256], BF16, kind="ExternalInput")
    wp_d = nc.dram_tensor("wp", [256, C], BF16, kind="ExternalInput")
    bqk_d = nc.dram_tensor("bqk", [128, 4], F32, kind="ExternalInput")
    bv_d = nc.dram_tensor("bv", [128, 256], BF16, kind="ExternalInput")
    idn_d = nc.dram_tensor("idn", [128, 128], BF16, kind="ExternalInput")
    mskb_d = nc.dram_tensor("mskb", [128, 128], BF16, kind="ExternalInput")
    out_d = nc.dram_tensor("out", [T, C], BF16, kind="ExternalOutput")

    Exp = mybir.ActivationFunctionType.Exp

    with tile.TileContext(nc) as tc:
        with (
            tc.tile_pool(name="const", bufs=1) as cpool,
            tc.tile_pool(name="qkT", bufs=1) as qkpool,
            tc.tile_pool(name="vbuf", bufs=1) as vpool,
            tc.tile_pool(name="ybuf", bufs=1) as ypool,
            tc.tile_pool(name="pt", bufs=4) as ptpool,
            tc.tile_pool(name="norm", bufs=2) as npool,
            tc.tile_pool(name="ostage", bufs=2) as opool,
            tc.tile_pool(name="mmps", bufs=2, space="PSUM") as mmps,
            tc.tile_pool(name="sps", bufs=2, space="PSUM") as sps,
            tc.tile_pool(name="accps", bufs=2, space="PSUM") as accps,
        ):
            # ---- constants / weights ----
            xT_s = cpool.tile([128, CT, T], BF16)
            wqk_s = cpool.tile([128, CT, 512], BF16)
            wv_s = cpool.tile([128, CT, 256], BF16)
            wp_s = cpool.tile([128, 2, C], BF16)
            bqk_s = cpool.tile([128, 4], F32)
            bv_s = cpool.tile([128, 256], BF16)
            idn_s = cpool.tile([128, 128], BF16)
            mskb_s = cpool.tile([128, 128], BF16)

            warm = cpool.tile([128, 1], F32)
            nc.vector.memset(warm[:], 0.0)
            nc.scalar.activation(warm[:], warm[:], Exp)

            for i in range(CT):
                nc.sync.dma_start(out=xT_s[:, i, 0:512],
                                  in_=xT_d.ap()[0, 128 * i:128 * (i + 1), :])
            for i in range(CT):
                nc.sync.dma_start(out=wqk_s[:, i, :], in_=wqk_d.ap()[128 * i:128 * (i + 1), :])
            nc.sync.dma_start(out=idn_s[:], in_=idn_d.ap()[:])
            nc.sync.dma_start(out=mskb_s[:], in_=mskb_d.ap()[:])
            nc.sync.dma_start(out=bqk_s[:], in_=bqk_d.ap()[:])
            for i in range(CT):
                nc.sync.dma_start(out=wv_s[:, i, :], in_=wv_d.ap()[128 * i:128 * (i + 1), :])
            nc.sync.dma_start(out=bv_s[:], in_=bv_d.ap()[:])
            for i in range(2):
                nc.sync.dma_start(out=wp_s[:, i, :], in_=wp_d.ap()[128 * i:128 * (i + 1), :])

            qkT_s = qkpool.tile([128, 4, T], BF16)
            v_s = vpool.tile([128, TQ, 4, 65], BF16)
            nc.vector.memset(v_s[:, :, :, 64:65], 1.0)
            yT_s = ypool.tile([128, 2, T], BF16)

            # ---- emission helpers (work groups used directly or as fillers) ----
            def dma_chunk(t4):
                for i in range(CT):
                    nc.sync.dma_start(out=xT_s[:, i, 512 * t4:512 * (t4 + 1)],
                                      in_=xT_d.ap()[t4, 128 * i:128 * (i + 1), :])

            Identity = mybir.ActivationFunctionType.Identity

            def qkv_half(j, t4, h):
                # jtile 0: Q heads {0,1}; 1: Q {2,3}; 2: K {0,1}; 3: K {2,3}
                # emitted as two filler halves (h=0 allocates, h=1 drains).
                # Early chunks drain on ACT (idle there); last chunk on DVE.
                ps = qkv_half.ps if h else mmps.tile([128, 512], F32, tag="mm")
                qkv_half.ps = ps
                for i in range(4 * h, 4 * h + 4):
                    nc.tensor.matmul(
                        ps[:],
                        wqk_s[:, i, 128 * j:128 * (j + 1)],
                        xT_s[:, i, 512 * t4:512 * (t4 + 1)],
                        start=(i == 0), stop=(i == CT - 1),
                    )
                if h:
                    dst = qkT_s[:, j, 512 * t4:512 * (t4 + 1)]
                    if t4 <= 2:
                        nc.scalar.activation(dst, ps[:], Identity,
                                             bias=bqk_s[:, j:j + 1])
                    else:
                        nc.vector.tensor_scalar_add(dst, ps[:], bqk_s[:, j:j + 1])

            def qkv_group(j, t4):
                qkv_half(j, t4, 0)
                qkv_half(j, t4, 1)

            def v_group(t):
                # bias lands via an identity-matmul accumulate (bv_s rows are
                # all bv), so the drain is a plain copy on ACT (early) or DVE.
                ps = mmps.tile([128, 256], F32, tag="mm")
                for i in range(CT):
                    nc.tensor.matmul(
                        ps[:],
                        xT_s[:, i, 128 * t:128 * (t + 1)],
                        wv_s[:, i, :],
                        start=(i == 0), stop=False,
                    )
                nc.tensor.matmul(ps[:], idn_s[:], bv_s[:],
                                 start=False, stop=True)
                dst = v_s[:, t, :, 0:64]
                src = ps[:].rearrange("p (h d) -> p h d", h=4)
                if t < 12:
                    nc.scalar.copy(dst, src)
                else:
                    nc.vector.tensor_copy(dst, src)

            def proj_half(t, n, o_t):
                ps = mmps.tile([128, 512], F32, tag="mm")
                for p2 in range(2):
                    nc.tensor.matmul(
                        ps[:],
                        yT_s[:, p2, 128 * t:128 * (t + 1)],
                        wp_s[:, p2, 512 * n:512 * (n + 1)],
                        start=(p2 == 0), stop=(p2 == 1),
                    )
                dst = o_t[:, 512 * n:512 * (n + 1)]
                if t >= 12:
                    nc.scalar.copy(dst, ps[:])
                else:
                    nc.vector.tensor_copy(dst, ps[:])
                if n == 1:
                    nc.sync.dma_start(out=out_d.ap()[128 * t:128 * (t + 1), :], in_=o_t[:])

            def proj_group(t):
                o_t = opool.tile([128, C], BF16, tag="o")
                proj_half(t, 0, o_t)
                proj_half(t, 1, o_t)

            def proj_fillers(t):
                """proj group as two filler-granular halves sharing one o_t."""
                box = {}

                def h(n, t=t, box=box):
                    if n == 0:
                        box["o"] = opool.tile([128, C], BF16, tag="o", name="o_t")
                    proj_half(t, n, box["o"])

                return [lambda: h(0), lambda: h(1)]

            NODL = (9, 9)
            fillers = deque()  # (deadline (qc, p), fn)

            def emit_filler(n=1):
                for _ in range(n):
                    if fillers:
                        fillers.popleft()[1]()

            def flush_due(key):
                """Emit every queued filler whose deadline is <= key."""
                keep = deque()
                while fillers:
                    dl, fn = fillers.popleft()
                    if dl <= key:
                        fn()
                    else:
                        keep.append((dl, fn))
                fillers.extend(keep)

            # ---- prologue: pair-0's Q/K j-tiles + V t-block 0..3 ----
            qkv_group(0, 0)
            qkv_group(2, 0)
            for t in range(4):
                v_group(t)

            # ---- attention: S^T tiles [k-block, q-chunk], flash over k ----
            for qc in range(QC):
                # stage work for later chunks (see scheduling notes in header)
                if qc == 0:
                    fillers.append(((0, 1), lambda: qkv_group(1, 0)))
                    fillers.append(((0, 1), lambda: qkv_group(3, 0)))
                if qc + 1 < QC:
                    dl = (qc + 1, 0)
                    fillers.append((dl, lambda t4=qc + 1: dma_chunk(t4)))
                    for j in range(4):
                        fillers.append((dl, lambda j=j, t4=qc + 1: qkv_half(j, t4, 0)))
                        fillers.append((dl, lambda j=j, t4=qc + 1: qkv_half(j, t4, 1)))
                    # V for the next chunk; the last chunk's V groups are
                    # deferred into qc3 itself (it is filler-starved).
                    if qc + 1 < QC - 1:
                        for t in range(4 * (qc + 1), 4 * (qc + 2)):
                            fillers.append((dl, lambda t=t: v_group(t)))
                if qc == QC - 1:
                    for t in range(4 * qc, 4 * (qc + 1)):
                        fillers.append(((qc, 1), lambda t=t: v_group(t)))
                # projection: qc0 during qc2; qc1+qc2 during qc3 (as halves)
                for tp in {2: range(0, 4), 3: range(4, 12)}.get(qc, ()):
                    for f in proj_fillers(tp):
                        fillers.append((NODL, f))

                for p in range(2):
                    flush_due((qc, p))
                    jq, jk = p, 2 + p
                    nkb = 4 * qc + 4
                    ya = accps.tile([65, 512], F32, tag="acc")
                    yb = accps.tile([65, 512], F32, tag="acc")
                    pts = {}

                    def stage(kb, qc=qc, jq=jq, jk=jk, pts=None):
                        """score (+ causal bias) matmuls + exp for both heads"""
                        m = kb - 4 * qc  # >= 0 on the diagonal chunk
                        lv = 128 * max(m, 0)  # first causally-live column
                        s_ps = sps.tile([128, 2, 512], F32, tag="spair")
                        for hi in range(2):
                            nc.tensor.matmul(
                                s_ps[:, hi, lv:512],
                                qkT_s[64 * hi:64 * (hi + 1), jk, 128 * kb:128 * (kb + 1)],
                                qkT_s[64 * hi:64 * (hi + 1), jq, 512 * qc + lv:512 * (qc + 1)],
                                start=True, stop=(m < 0),
                                tile_position=(64 * hi, 0), skip_group_check=True)
                        if m >= 0:
                            for hi in range(2):
                                nc.tensor.matmul(
                                    s_ps[:, hi, lv:lv + 128],
                                    idn_s[:], mskb_s[:],
                                    start=False, stop=True,
                                    tile_position=(0, 0), skip_group_check=True)
                        pt = ptpool.tile([128, 2, 512], BF16, tag="pt")
                        nc.scalar.activation(pt[:, :, lv:512], s_ps[:, :, lv:512],
                                             Exp, scale=SCALE)
                        pts[kb] = (pt, lv)

                    def consume(kb, p=p, ya=ya, yb=yb, nkb=nkb, qc=qc, pts=None):
                        pt, lv = pts.pop(kb)
                        for hi, y_ps in ((0, ya), (1, yb)):
                            nc.tensor.matmul(
                                y_ps[:, lv:512],
                                v_s[:, kb, 2 * p + hi, :],
                                pt[:, hi, lv:512],
                                start=(kb == 0), stop=(kb == nkb - 1),
                                skip_group_check=True)

                    DEPTH = 2
                    for kb in range(min(DEPTH, nkb)):
                        stage(kb, pts=pts)
                    for kb in range(nkb):
                        if kb + DEPTH < nkb:
                            stage(kb + DEPTH, pts=pts)
                        consume(kb, pts=pts)
                        emit_filler(2 if qc == 0 else 1)

                    # normalize + write y^T (head A -> partitions 0:64, B -> 64:128).
                    # Copies run inline (they free the accumulator PSUM banks);
                    # reciprocal -> broadcast DMA -> multiply are deferred into
                    # the next attention steps.
                    yc = npool.tile([65, 2, 512], F32, tag="yc")
                    nc.vector.tensor_copy(yc[:, 0, :], ya[:])
                    nc.vector.tensor_copy(yc[:, 1, :], yb[:])
                    rb = npool.tile([64, 2, 512], F32, tag="rb")
                    rr = npool.tile([64, 2, 512], F32, tag="rr")

                    def norm_bcast(yc=yc, rb=rb):
                        # DMA (not DVE) reads the partition-64 denominator row:
                        # DVE input partition offsets are unreliable on HW.
                        nc.sync.dma_start(
                            out=rb[:],
                            in_=yc[64:65, :, :].unsqueeze(1)
                                .broadcast_to([1, 64, 2, 512]))

                    def norm_recip(rb=rb, rr=rr):
                        nc.vector.reciprocal_approx_fast(rr[:], rb[:])

                    def norm_mul(p=p, qc=qc, yc=yc, rr=rr):
                        # gpsimd (otherwise idle; SBUF-only operands)
                        for hi in range(2):
                            nc.gpsimd.tensor_mul(
                                yT_s[64 * hi:64 * (hi + 1), p, 512 * qc:512 * (qc + 1)],
                                yc[0:64, hi, :], rr[:, hi, :])

                    fillers.appendleft((NODL, norm_mul))
                    fillers.appendleft((NODL, norm_recip))
                    fillers.appendleft((NODL, norm_bcast))

            # ---- epilogue: leftover fillers (incl. last norm chain) + final
            # projection chunk (ACT drains; ACT is idle here) ----
            emit_filler(len(fillers))
            if dbg:
                nc.sync.dma_start(out=dbg_qkT.ap()[:], in_=qkT_s[:])
                nc.sync.dma_start(out=dbg_v.ap()[:], in_=v_s[:])
                nc.sync.dma_start(out=dbg_yT.ap()[:], in_=yT_s[:])
            for t in range(4 * (QC - 1), TQ):
                proj_group(t)

    nc.compile()
    return nc


def _shard_inputs(x, W_attn, b_attn, W_proj, b_proj):
    """Build the 8 per-core input maps (numpy, bf16 where applicable)."""
    pp = np.arange(128)[:, None]
    jj = np.arange(128)[None, :]
    mskb = np.where(pp > jj, NEG, 0.0).astype(NP_BF16)  # [128, 128]
    idn = np.eye(128, dtype=NP_BF16)
    in_maps = []
    for c in range(N_CORES):
        b, g = c // 4, c % 4
        ch = slice(256 * g, 256 * (g + 1))
        wq = W_attn[:, ch]
        wk = W_attn[:, C:][:, ch]
        wv = W_attn[:, 2 * C:][:, ch]
        wqk = np.concatenate([wq, wk], axis=1).astype(NP_BF16)
        bq = b_attn[ch]
        bk = b_attn[C:][ch]
        bv = b_attn[2 * C:][ch]
        bqk = np.concatenate([bq, bk]).reshape(4, 128).T.astype(np.float32)  # [128, 4]
        xTc = np.ascontiguousarray(
            x[b].T.reshape(C, QC, 512).transpose(1, 0, 2)).astype(NP_BF16)
        in_maps.append({
            "xT": xTc,
            "wqk": wqk,
            "wv": wv.astype(NP_BF16),
            "wp": W_proj[ch, :].astype(NP_BF16),
            "bqk": np.ascontiguousarray(bqk),
            "bv": np.broadcast_to(bv.astype(NP_BF16), (128, 256)).copy(),
            "idn": idn,
            "mskb": mskb,
        })
    return in_maps


def _run(in_maps, trace=False, **kw):
    global _compiled
    if _compiled is None:
        _compiled = _build_nc()
    return run_bass_kernel_spmd(_compiled, in_maps, list(range(N_CORES)),
                                trace=trace, **kw)


def kernel(x, W_attn, b_attn, W_proj, b_proj):
    x = np.asarray(x, dtype=np.float32)
    W_attn = np.asarray(W_attn, dtype=np.float32)
    b_attn = np.asarray(b_attn, dtype=np.float32)
    W_proj = np.asarray(W_proj, dtype=np.float32)
    b_proj = np.asarray(b_proj, dtype=np.float32)

    in_maps = _shard_inputs(x, W_attn, b_attn, W_proj, b_proj)
    res = _run(in_maps)
    out = np.zeros((B, T, C), dtype=np.float32)
    for c in range(N_CORES):
        out[c // 4] += np.asarray(res.results[c]["out"], dtype=np.float32)
    out += b_proj
    return out
